# revision 1
# baseline (speedup 1.0000x reference)
"""Trainium2 Bass kernel for nn_Attention_3d (B=1, C=64, D=48, W=128, H=128, 8 heads).

Sharding: depth D split across 8 cores (6 slices each), weights replicated.

Per-core pipeline (per depth slice d):
  1. QKV conv as a "transposed conv": stationary = x w-pair chunks
     [c-pair(128) x h(128)] (host interleaves w-even/w-odd channels on
     partitions 0-63/64-127), moving = blockdiag([wq|wk|wv]^T) [128,384],
     fp32r -> PSUM [h, 384] per w-pair -> evac:
       - Q,K (+bias) via DVE tensor_add -> qkbuf [h, qk, c, w] fp32r
       - V -> vT tiles [h, c, w] bf16 (ScalarE copy)
  2. V tiles PE-transposed -> v_nat [w, c, 129] fp32r (col 128 = ones).
  3. Per channel pair: scoresT = kT^T @ [qT_c|qT_c'] (fp32r, N=256,
     half-garbage), exp(x - 80) on ScalarE (shift cancels in softmax,
     avoids fp32 overflow) -> ET [v,w] fp32r,
     AV paired: o = ET^T @ [v_c|1|v_c'|1] fp32r -> PSUM [w, 258] (col 128 =
     softmax denominator), reciprocal + per-partition scale on evac ->
     obuf [w, h, c] fp32r.
  4. O regroup: PE-transpose [w, (h-pair, c)] blocks -> [c-pair, w] chunks,
     final conv with blockdiag(wo^T) fp32r, bias bo' = bo + wo@bv (v-bias
     folded analytically), -> Y groups DMA'd in a device-friendly layout,
     decoded on host.
"""
import time
import numpy as np
import ml_dtypes
from contextlib import ExitStack

B, C, D, W, H = 1, 64, 48, 128, 128
NCORES = 8
DLOC = D // NCORES  # 6
QSCALE = float(8.0 ** -0.25)

_f32 = np.float32
_bf16 = ml_dtypes.bfloat16
DEBUG = False


def round_fp32r(x):
    u = np.ascontiguousarray(x, dtype=np.float32).view(np.uint32)
    lsb = (u >> 12) & 1
    u = (u + 0x7FF + lsb) & np.uint32(0xFFFFF000)
    return u.view(np.float32)


def _build_bass():
    import concourse.bacc as bacc
    import concourse.bass as bass
    import concourse.mybir as mybir
    import concourse.tile as tile

    fr = mybir.dt.float32r
    f32 = mybir.dt.float32
    bf = mybir.dt.bfloat16
    fh = mybir.dt.float16
    AF = mybir.ActivationFunctionType

    nc = bacc.Bacc("TRN2", target_bir_lowering=False, debug=False)

    xp_d = nc.dram_tensor("xp", [128, DLOC * 64 * 128], fr, kind="ExternalInput").ap()
    wbd_d = nc.dram_tensor("wbd", [128, 384], fr, kind="ExternalInput").ap()
    bqk_d = nc.dram_tensor("bqk", [128, 2 * 2 * 128], f32, kind="ExternalInput").ap()
    ident_d = nc.dram_tensor("ident", [128, 128], bf, kind="ExternalInput").ap()
    wobd_d = nc.dram_tensor("wobd", [128, 128], fr, kind="ExternalInput").ap()
    identr_d = nc.dram_tensor("identr", [128, 128], fr, kind="ExternalInput").ap()
    bo2_d = nc.dram_tensor("bo2", [128, 1], f32, kind="ExternalInput").ap()
    y_d = nc.dram_tensor("y", [DLOC, 16, 128, 512], f32, kind="ExternalOutput").ap()
    if DEBUG:
        dbg_qk = nc.dram_tensor("dbg_qk", [128, 2 * 64 * 128], f32, kind="ExternalOutput").ap()
        dbg_vb = nc.dram_tensor("dbg_vb", [128, 64 * 129], bf, kind="ExternalOutput").ap()
        dbg_ob = nc.dram_tensor("dbg_ob", [128, 128 * 64], bf, kind="ExternalOutput").ap()
        dbg_sc = nc.dram_tensor("dbg_sc", [128, 512], f32, kind="ExternalOutput").ap()

    with tile.TileContext(nc) as tc, ExitStack() as ctx:
        const = ctx.enter_context(tc.tile_pool(name="const", bufs=1))
        xpool = ctx.enter_context(tc.tile_pool(name="xpool", bufs=1))
        qkpool = ctx.enter_context(tc.tile_pool(name="qkpool", bufs=1))
        vtpool = ctx.enter_context(tc.tile_pool(name="vtpool", bufs=1))
        vbpool = ctx.enter_context(tc.tile_pool(name="vbpool", bufs=1))
        etpool = ctx.enter_context(tc.tile_pool(name="etpool", bufs=3))
        rcpool = ctx.enter_context(tc.tile_pool(name="rcpool", bufs=6))
        obpool = ctx.enter_context(tc.tile_pool(name="obpool", bufs=1))
        ochpool = ctx.enter_context(tc.tile_pool(name="ochpool", bufs=3))
        ypool = ctx.enter_context(tc.tile_pool(name="ypool", bufs=3))

        pconv = ctx.enter_context(tc.tile_pool(name="pconv", bufs=2, space="PSUM"))
        pscp = ctx.enter_context(tc.tile_pool(name="pscp", bufs=2, space="PSUM"))
        pavp = ctx.enter_context(tc.tile_pool(name="pavp", bufs=1, space="PSUM"))
        ptrp = ctx.enter_context(tc.tile_pool(name="ptrp", bufs=1, space="PSUM"))
        pyp = ctx.enter_context(tc.tile_pool(name="pyp", bufs=1, space="PSUM"))

        # constants
        wbd_sb = const.tile([128, 384], fr)
        nc.gpsimd.dma_start(wbd_sb[:], wbd_d[:])
        bqk_sb = const.tile([128, 2, 2, 128], f32)
        nc.gpsimd.dma_start(
            bqk_sb[:], bqk_d[:].rearrange("p (a b c) -> p a b c", a=2, b=2)
        )
        ident_sb = const.tile([128, 128], bf)
        nc.gpsimd.dma_start(ident_sb[:], ident_d[:])
        wobd_sb = const.tile([128, 128], fr)
        nc.gpsimd.dma_start(wobd_sb[:], wobd_d[:])
        identr_sb = const.tile([128, 128], fr)
        nc.gpsimd.dma_start(identr_sb[:], identr_d[:])
        eshift_sb = const.tile([128, 1], f32)
        nc.gpsimd.memset(eshift_sb[:], -80.0)
        bo2_sb = const.tile([128, 1], f32)
        nc.gpsimd.dma_start(bo2_sb[:], bo2_d[:])
        ones_bf = const.tile([128, 64], bf)
        nc.gpsimd.memset(ones_bf[:], 1.0)


        for d in range(DLOC):
            xp_sb = xpool.tile([128, 64 * 128], fr)
            nc.gpsimd.dma_start(
                xp_sb[:], xp_d[:, d * 8192:(d + 1) * 8192]
            )

            qkbuf = qkpool.tile([128, 2, 64, 128], fr)   # [h, qk, c, w]
            vtbuf = vtpool.tile([128, 64, 128], bf)      # [h, c, w]

            # ---- stage 1: QKV conv sweep (w-pairs, 2 pairs per psum tile)
            for wp in range(64):
                pc = pconv.tile([128, 512], f32)
                nc.tensor.matmul(
                    pc[:, 0:384],
                    xp_sb[:, wp * 128:(wp + 1) * 128],
                    wbd_sb[:],
                    start=True, stop=True,
                )
                src = pc[:, 0:384].rearrange("p (w2 c) -> p w2 c", w2=2)
                # Q,K part (+bias), fp32r out
                dst_qk = qkbuf[:, :, :, 2 * wp:2 * wp + 2].rearrange(
                    "p qk c w2 -> p w2 (qk c)"
                )
                nc.vector.tensor_add(dst_qk, src[:, :, 0:128], bqk_sb[:, 0])
                # V part, bf16 out
                dst_v = vtbuf[:, :, 2 * wp:2 * wp + 2].rearrange(
                    "p c w2 -> p w2 c"
                )
                nc.scalar.copy(dst_v, src[:, :, 128:192])

            # ---- stage 2: V transposes -> v_nat
            vbuf = vbpool.tile([128, 64, 129], fr)       # [w(v), c, h+ones]
            nc.vector.tensor_copy(vbuf[:, :, 128], ones_bf[:])
            for c4 in range(0, 64, 4):
                pt = ptrp.tile([128, 512], bf, tag="pt")
                for j in range(4):
                    nc.tensor.transpose(
                        pt[:, j * 128:(j + 1) * 128],
                        vtbuf[:, c4 + j, :], ident_sb[:],
                    )
                nc.vector.tensor_copy(
                    vbuf[:, c4:c4 + 4, 0:128],
                    pt[:].rearrange("p (a b) -> p a b", a=4),
                )

            # ---- stage 3: attention per channel pair
            obuf = obpool.tile([128, 128, 64], fr)       # [w, h, c]
            for c2 in range(0, 64, 2):
                psc = pscp.tile([128, 2, 256], f32)      # one bank
                flat = psc[:].rearrange("p a b -> p (a b)")
                rhs_pair = qkbuf[:, 0, c2:c2 + 2, :].rearrange("p c w -> p (c w)")
                nc.tensor.matmul(
                    psc[:, 0, 0:256], qkbuf[:, 1, c2, :], rhs_pair,
                    start=True, stop=True,
                )
                nc.tensor.matmul(
                    flat[:, 128:384], qkbuf[:, 1, c2 + 1, :], rhs_pair,
                    start=True, stop=True,
                )
                if DEBUG and d == 0 and c2 == 0:
                    scf = ypool.tile([128, 512], f32, tag="dbgs")
                    nc.vector.tensor_copy(scf[:], psc[:].rearrange("p a b -> p (a b)"))
                    nc.gpsimd.dma_start(dbg_sc[:], scf[:])
                et = etpool.tile([128, 2, 128], fr)
                nc.scalar.activation(et[:], psc[:, :, 0:128], AF.Exp,
                                     bias=eshift_sb[:])

                pav = pavp.tile([128, 2, 512], f32)
                rhs_av = vbuf[:, c2:c2 + 2, :].rearrange("p c h -> p (c h)")
                for j in (0, 1):
                    nc.tensor.matmul(
                        pav[:, j, 0:258], et[:, j, :], rhs_av,
                        start=True, stop=True,
                    )
                rc = rcpool.tile([128, 2], f32)
                nc.scalar.copy(rc[:], pav[:, :, 128])
                rr = rcpool.tile([128, 2], f32, tag="rr")
                nc.vector.reciprocal(rr[:], rc[:])
                nc.vector.tensor_scalar_mul(
                    obuf[:, :, c2], pav[:, 0, 0:128], rr[:, 0:1]
                )
                nc.vector.tensor_scalar_mul(
                    obuf[:, :, c2 + 1], pav[:, 1, 129:257], rr[:, 1:2]
                )

            if DEBUG and d == 0:
                qkf = qkbuf[:].rearrange("p a c w -> p (a c w)").bitcast(f32)
                nc.gpsimd.dma_start(dbg_qk[:], qkf)
                nc.gpsimd.dma_start(dbg_vb[:], vbuf[:].rearrange("p a c -> p (a c)"))
                nc.gpsimd.dma_start(dbg_ob[:], obuf[:].rearrange("p a c -> p (a c)"))

            # ---- stage 4: O regroup + final conv
            for g in range(16):
                pt = ptrp.tile([128, 512], fr, tag="pt")
                for t in range(4):
                    hp = g * 4 + t
                    nc.tensor.transpose(
                        pt[:, t * 128:(t + 1) * 128],
                        obuf[:, 2 * hp:2 * hp + 2, :].rearrange("p a c -> p (a c)"),
                        identr_sb[:],
                    )
                och = ochpool.tile([128, 512], fr)
                nc.vector.tensor_copy(och[:], pt[:])
                py = pyp.tile([128, 512], f32)
                nc.tensor.matmul(py[:], wobd_sb[:], och[:], start=True, stop=True)
                yb = ypool.tile([128, 512], f32)
                nc.scalar.activation(
                    yb[:], py[:], AF.Identity, bias=bo2_sb[:]
                )
                nc.gpsimd.dma_start(y_d[d, g], yb[:])

    nc.compile()
    return nc


def _prep_inputs(x, wq, bq, wk, bk, wv, bv, wo, bo):
    """Build per-core input maps."""
    x = np.asarray(x, _f32)[0]           # [64, 48, 128, 128]
    wq2 = np.asarray(wq, _f32) * QSCALE
    bq2 = np.asarray(bq, _f32) * QSCALE
    wk = np.asarray(wk, _f32); bk = np.asarray(bk, _f32)
    wv = np.asarray(wv, _f32); bv = np.asarray(bv, _f32)
    wo = np.asarray(wo, _f32); bo = np.asarray(bo, _f32)

    A = np.zeros((64, 192), _f32)
    A[:, 0:64] = wq2.T
    A[:, 64:128] = wk.T
    A[:, 128:192] = wv.T
    wbd = np.zeros((128, 384), _f32)
    wbd[0:64, 0:192] = A
    wbd[64:128, 192:384] = A
    wbd = round_fp32r(wbd)

    bqk_row = np.concatenate([bq2, bk])            # [128]
    bqk = np.ascontiguousarray(
        np.broadcast_to(bqk_row[None, None, :], (128, 4, 128)).reshape(128, 512),
        dtype=_f32,
    )

    ident = np.eye(128, dtype=_f32).astype(_bf16)

    wobd = np.zeros((128, 128), _f32)
    wobd[0:64, 0:64] = wo.T
    wobd[64:128, 64:128] = wo.T
    wobd = round_fp32r(wobd)
    identr = round_fp32r(np.eye(128, dtype=_f32))

    bo2v = (bo + wo @ bv).astype(_f32)
    bo2 = np.concatenate([bo2v, bo2v]).reshape(128, 1)

    in_maps = []
    for i in range(NCORES):
        xc = x[:, i * DLOC:(i + 1) * DLOC]          # [64, 6, 128, 128]
        xp = np.empty((128, DLOC, 64, 128), _f32)
        xp[0:64] = xc[:, :, 0::2, :]                # w even
        xp[64:128] = xc[:, :, 1::2, :]              # w odd
        xp = round_fp32r(xp.reshape(128, DLOC * 64 * 128))
        in_maps.append({
            "xp": xp, "wbd": wbd, "bqk": bqk, "ident": ident,
            "identr": identr, "wobd": wobd, "bo2": bo2,
        })
    return in_maps


def _decode_outputs(results):
    """results: list of per-core dicts with 'y' [6, 16, 128, 512]."""
    outs = []
    for r in results:
        y = np.asarray(r["y"], _f32).reshape(DLOC, 16, 2, 64, 4, 128)
        # dims: [d, g, ph, c, t, w] ; h = g*8 + t*2 + ph
        Y = np.transpose(y, (3, 0, 5, 1, 4, 2)).reshape(64, DLOC, 128, 128)
        outs.append(Y)
    return np.concatenate(outs, axis=1)[None]        # [1, 64, 48, 128, 128]


_CACHE = {}


def _get_runner():
    """Build bass program + cached jitted pjrt callable."""
    if "runner" in _CACHE:
        return _CACHE["runner"]
    import jax
    import jax.numpy as jnp
    from jax.sharding import Mesh, PartitionSpec
    from jax.experimental.shard_map import shard_map
    import concourse.mybir as mybir
    from concourse import bass2jax
    from concourse.bass2jax import _bass_exec_p, install_neuronx_cc_hook

    install_neuronx_cc_hook()
    nc = _build_bass()

    partition_name = (
        nc.partition_id_tensor.name if nc.partition_id_tensor else None
    )
    in_names, out_names, out_avals = [], [], []
    for alloc in nc.m.functions[0].allocations:
        if not isinstance(alloc, mybir.MemoryLocationSet):
            continue
        name = alloc.memorylocations[0].name
        if alloc.kind == "ExternalInput":
            if name != partition_name:
                in_names.append(name)
        elif alloc.kind == "ExternalOutput":
            out_names.append(name)
            out_avals.append(
                jax.core.ShapedArray(
                    tuple(alloc.tensor_shape), mybir.dt.np(alloc.dtype)
                )
            )
    n_params = len(in_names)
    zero_shapes = [(a.shape, a.dtype) for a in out_avals]
    all_in_names = list(in_names) + list(out_names)
    if partition_name is not None:
        all_in_names.append(partition_name)

    def _body(*args):
        operands = list(args)
        if partition_name is not None:
            operands.append(bass2jax.partition_id_tensor())
        outs = _bass_exec_p.bind(
            *operands,
            out_avals=tuple(out_avals),
            in_names=tuple(all_in_names),
            out_names=tuple(out_names),
            lowering_input_output_aliases=(),
            sim_require_finite=True,
            sim_require_nnan=True,
            nc=nc,
        )
        return tuple(outs)

    devices = jax.devices()[:NCORES]
    mesh = Mesh(np.asarray(devices), ("core",))
    n_outs = len(out_names)
    in_specs = (PartitionSpec("core"),) * (n_params + n_outs)
    out_specs = (PartitionSpec("core"),) * n_outs
    donate = tuple(range(n_params, n_params + n_outs))
    sharded = jax.jit(
        shard_map(_body, mesh=mesh, in_specs=in_specs, out_specs=out_specs,
                  check_rep=False),
        donate_argnums=donate,
        keep_unused=True,
    )

    def run(in_maps):
        concat_in = [
            np.concatenate([np.asarray(in_maps[c][nm]) for c in range(NCORES)],
                           axis=0)
            for nm in in_names
        ]
        concat_zeros = [
            np.zeros((NCORES * s[0],) + tuple(s[1:]), dt)
            for (s, dt) in zero_shapes
        ]
        out = sharded(*concat_in, *concat_zeros)
        res = []
        for c in range(NCORES):
            res.append({
                nm: np.asarray(out[i]).reshape(NCORES, *zero_shapes[i][0])[c]
                for i, nm in enumerate(out_names)
            })
        return res, (sharded, in_names, zero_shapes, out_names)

    _CACHE["runner"] = run
    return run


def kernel(**inputs):
    run = _get_runner()
    in_maps = _prep_inputs(**inputs)
    results, _ = run(in_maps)
    return _decode_outputs(results)


if __name__ == "__main__":
    import reference
    t0 = time.time()
    ins = {k: np.asarray(v) for k, v in reference.setup_inputs().items()}
    exp = np.asarray(reference.reference(**ins))
    t1 = time.time()
    print(f"reference: {t1 - t0:.1f}s", flush=True)
    act = kernel(**ins)
    t2 = time.time()
    print(f"kernel: {t2 - t1:.1f}s", flush=True)
    err = np.abs(act - exp)
    scale = np.abs(exp).mean()
    print(f"abs err max={err.max():.3e} mean={err.mean():.3e} "
          f"rel(max/scale)={err.max() / scale:.3e} "
          f"rel_mean={(err / (np.abs(exp) + 1e-6)).mean():.3e}", flush=True)



# revision 43
# speedup vs baseline: 2.0887x; 2.0887x over previous
"""Trainium2 Bass kernel for nn_Attention_3d (B=1, C=64, D=48, W=128, H=128, 8 heads).

Sharding: depth D split across 8 cores (6 slices each), weights replicated.

v2 design (evacuation-balanced; cost-model ~340us/core vs 711us baseline):
  - conv-A (Q,K only): stationary = x w-parity pair chunks [c-pair(128) x h(128)]
    fp32r, moving = blockdiag([wq'|wk]^T) fp32r [128,256] -> PSUM per wp.
    Evac: Q via DVE tensor_add(+bq') -> qbuf fp16; K via ScalarE copy -> kbuf
    fp16.  (bk is dropped entirely: it only shifts scores by a per-query
    constant, which softmax cancels; bv is folded into the output bias.)
  - conv-B (V, produced pre-transposed): x is ALSO loaded in an h-parity
    layout (fp16), stationary = [c-pair(h-par) x w(128)], moving =
    blockdiag(wv^T) fp16 [128,128] -> PSUM [w, (h-par, c)] per h-pair ->
    evac straight into v_nat [w, c, h+ones] fp32r.  This kills the separate
    V-natural evac + PE transposes of the original design.
  - scores: fp16 matmuls at N=128 (no fp32r>=256 constraint), 4 channels
    per PSUM bank, one FD=512 exp(x-80) on ScalarE -> et fp32r (the -80
    shift keeps exp finite; denominators renormalize it away).
  - AV (fp32r): both channels of a pair in ONE psum bank: mm0 -> [0:258]
    (o|denom|garbage), mm1 -> [256:512] with rhs starting at channel c+1's
    v-block, so useful cols sit at uniform stride; rhs = flat v_nat with
    interleaved ones columns.  reciprocal(denoms) on DVE; o evac scales
    during PSUM evac (values underflow fp16 unscaled): half direct DVE
    tensor_scalar, half ScalarE-staged fp32 + DVE 2x-mode tensor_scalar
    -> obuf fp16 [w, h, c].
  - out: PE-transpose obuf fp16 blocks [w, (2h,c)] -> [(hp,c), w] psum fp16
    (1 cyc/row), evac fp16->fp16 (DVE 2x mode) -> och, final conv fp16
    blockdiag(wo^T) N=512, bias bo' = bo + wo@bv on ScalarE evac, y DMA
    per 2048 cols.  Stage D of slice d is emitted after conv stages of d+1.
  - Pipelining: qbuf/kbuf/obuf double-buffered (conv-A/Q/K evacs of d+1
    overlap attention of d); input DMAs chunked 4x; const loads on the SP
    DMA queue; PSUM: pA2/pCB1/pSC1/pAV2/pOT1/pPY1 banks.
"""
import time
import numpy as np
import ml_dtypes
from contextlib import ExitStack

B, C, D, W, H = 1, 64, 48, 128, 128
NCORES = 8
DLOC = D // NCORES  # 6
QSCALE = float(8.0 ** -0.25)

_f32 = np.float32
_f16 = np.float16
DEBUG = False


def round_fp32r(x):
    u = np.ascontiguousarray(x, dtype=np.float32).view(np.uint32)
    lsb = (u >> 12) & 1
    u = (u + 0x7FF + lsb) & np.uint32(0xFFFFF000)
    return u.view(np.float32)


def _build_bass():
    import concourse.bacc as bacc
    import concourse.bass as bass
    import concourse.mybir as mybir
    import concourse.tile as tile

    fr = mybir.dt.float32r
    f32 = mybir.dt.float32
    fh = mybir.dt.float16
    AF = mybir.ActivationFunctionType

    nc = bacc.Bacc("TRN2", target_bir_lowering=False, debug=False)

    xa_d = nc.dram_tensor("xa", [128, DLOC * 64 * 128], fr, kind="ExternalInput").ap()
    xb_d = nc.dram_tensor("xb", [128, DLOC * 64 * 128], fh, kind="ExternalInput").ap()
    wbda_d = nc.dram_tensor("wbda", [128, 256], fr, kind="ExternalInput").ap()
    wvbd_d = nc.dram_tensor("wvbd", [128, 128], fh, kind="ExternalInput").ap()
    bq4_d = nc.dram_tensor("bq4", [128, 256], f32, kind="ExternalInput").ap()
    identh_d = nc.dram_tensor("identh", [128, 128], fh, kind="ExternalInput").ap()
    wobd2_d = nc.dram_tensor("wobd2", [128, 128], fh, kind="ExternalInput").ap()
    bo2i_d = nc.dram_tensor("bo2i", [128, 1], f32, kind="ExternalInput").ap()
    y_d = nc.dram_tensor("y", [DLOC, 4, 128, 2048], f32, kind="ExternalOutput").ap()
    if DEBUG:
        dbg_qk = nc.dram_tensor("dbg_qk", [128, 2 * 64 * 128], fh, kind="ExternalOutput").ap()
        dbg_vb = nc.dram_tensor("dbg_vb", [128, 64 * 129], f32, kind="ExternalOutput").ap()
        dbg_ob = nc.dram_tensor("dbg_ob", [128, 64 * 128], fh, kind="ExternalOutput").ap()
        dbg_sc = nc.dram_tensor("dbg_sc", [128, 512], f32, kind="ExternalOutput").ap()

    with tile.TileContext(nc) as tc, ExitStack() as ctx:
        const = ctx.enter_context(tc.tile_pool(name="const", bufs=1))
        xapool = ctx.enter_context(tc.tile_pool(name="xapool", bufs=1))
        xbpool = ctx.enter_context(tc.tile_pool(name="xbpool", bufs=1))
        qkpool = ctx.enter_context(tc.tile_pool(name="qkpool", bufs=2))
        vbpool = ctx.enter_context(tc.tile_pool(name="vbpool", bufs=1))
        etpool = ctx.enter_context(tc.tile_pool(name="etpool", bufs=2))
        rcpool = ctx.enter_context(tc.tile_pool(name="rcpool", bufs=6))
        obpool = ctx.enter_context(tc.tile_pool(name="obpool", bufs=2))
        ochpool = ctx.enter_context(tc.tile_pool(name="ochpool", bufs=2))
        ypool = ctx.enter_context(tc.tile_pool(name="ypool", bufs=2))
        ostgpool = ctx.enter_context(tc.tile_pool(name="ostgpool", bufs=3))

        # PSUM: 8 banks total (bufs are per-tag/pool)
        pA = ctx.enter_context(tc.tile_pool(name="pA", bufs=2, space="PSUM"))
        pCB = ctx.enter_context(tc.tile_pool(name="pCB", bufs=1, space="PSUM"))
        pSC = ctx.enter_context(tc.tile_pool(name="pSC", bufs=1, space="PSUM"))
        pAV = ctx.enter_context(tc.tile_pool(name="pAV", bufs=2, space="PSUM"))
        pOT = ctx.enter_context(tc.tile_pool(name="pOT", bufs=1, space="PSUM"))
        pPY = ctx.enter_context(tc.tile_pool(name="pPY", bufs=1, space="PSUM"))

        # constants: memsets on DVE, const loads on the idle SP DMA queue
        eshift_sb = const.tile([128, 1], f32)
        nc.vector.memset(eshift_sb[:], -80.0)
        ones_f = const.tile([128, 64], f32)
        nc.vector.memset(ones_f[:], 1.0)
        wvbd_sb = const.tile([128, 128], fh)
        nc.sync.dma_start(wvbd_sb[:], wvbd_d[:])
        wbda_sb = const.tile([128, 256], fr)
        nc.sync.dma_start(wbda_sb[:], wbda_d[:])
        bq4_sb = const.tile([128, 2, 2, 64], f32)
        nc.sync.dma_start(
            bq4_sb[:], bq4_d[:].rearrange("p (a b c) -> p a b c", a=2, b=2)
        )
        identh_sb = const.tile([128, 128], fh)
        nc.sync.dma_start(identh_sb[:], identh_d[:])
        wobd2_sb = const.tile([128, 128], fh)
        nc.sync.dma_start(wobd2_sb[:], wobd2_d[:])
        bo2i_sb = const.tile([128, 1], f32)
        nc.sync.dma_start(bo2i_sb[:], bo2i_d[:])

        def stage_d(d, obuf):
            # ---- stage D: O transpose + final conv, 8 groups of 8 h-pairs
            for q8 in range(8):
                if q8 % 2 == 0:
                    yb = ypool.tile([128, 4, 512], f32, name="yb")
                pt = pOT.tile([128, 1024], fh, name="pt")
                for t in range(8):
                    hp = 8 * q8 + t
                    nc.tensor.transpose(
                        pt[:, t * 128:(t + 1) * 128],
                        obuf[:, 2 * hp:2 * hp + 2, :].rearrange(
                            "p a c -> p (a c)"),
                        identh_sb[:],
                    )
                och = ochpool.tile([128, 1024], fh, name="och")
                nc.vector.tensor_copy(och[:], pt[:])
                for half in range(2):
                    py = pPY.tile([128, 512], f32, name="py")
                    nc.tensor.matmul(
                        py[:], wobd2_sb[:], och[:, half * 512:(half + 1) * 512],
                        start=True, stop=True)
                    nc.scalar.activation(
                        yb[:, 2 * (q8 % 2) + half, :], py[:], AF.Identity,
                        bias=bo2i_sb[:])
                if q8 % 2 == 1:
                    nc.gpsimd.dma_start(
                        y_d[d, q8 // 2], yb[:].rearrange("p a b -> p (a b)")
                    )

        prev = [None]

        for d in range(DLOC):
            # ---- input loads
            xa_sb = xapool.tile([128, 64 * 128], fr)
            xb_sb = xbpool.tile([128, 64 * 128], fh)
            for ch in range(4):
                lo, hi = ch * 2048, (ch + 1) * 2048
                nc.gpsimd.dma_start(xb_sb[:, lo:hi], xb_d[:, d * 8192 + lo:d * 8192 + hi])
                nc.gpsimd.dma_start(xa_sb[:, lo:hi], xa_d[:, d * 8192 + lo:d * 8192 + hi])

            # ---- stage A: conv-B -> v_nat [w, c, h] fp32r (+ ones col 128)
            # flat [128, 64*129 + 126] with tail pad so the last AV mm's
            # 256-wide moving slice stays in bounds.
            vflat = vbpool.tile([128, 64 * 129 + 128], fr)
            vbuf = vflat[:, 0:64 * 129].rearrange("p (c h) -> p c h", c=64)
            nc.gpsimd.memset(vflat[:, 64 * 129:].bitcast(f32), 0.0)
            nc.vector.tensor_copy(vbuf[:, :, 128], ones_f[:])
            for g in range(16):
                pbt = pCB.tile([128, 512], f32)
                pb = pbt[:].rearrange("p (a b) -> p a b", a=4)
                for t in range(4):
                    hp = 4 * g + t
                    nc.tensor.matmul(
                        pb[:, t, :],
                        xb_sb[:, hp * 128:(hp + 1) * 128],
                        wvbd_sb[:],
                        start=True, stop=True,
                    )
                # src [w, t(4), par(2), c(64)] -> dst vbuf[w, c, 8h]
                src = pb[:].rearrange("p t (r c) -> p c (t r)", r=2)
                if g % 2 == 0:
                    nc.vector.tensor_copy(vbuf[:, :, 8 * g:8 * g + 8], src)
                else:
                    nc.scalar.copy(vbuf[:, :, 8 * g:8 * g + 8], src)

            # ---- stage B: conv-A -> qbuf, kbuf fp16 [h, c, w]
            qbuf = qkpool.tile([128, 64, 128], fh, tag="q")
            kbuf = qkpool.tile([128, 64, 128], fh, tag="k")
            for g in range(32):
                pat = pA.tile([128, 512], f32)
                # [h, wp(2), par(2), qk*64]
                pa = pat[:].rearrange("p (a b c) -> p a b c", a=2, b=2)
                for i in range(2):
                    wp = 2 * g + i
                    nc.tensor.matmul(
                        pa[:, i, :, :].rearrange("p a b -> p (a b)"),
                        xa_sb[:, wp * 128:(wp + 1) * 128],
                        wbda_sb[:],
                        start=True, stop=True,
                    )
                # cols per parity: [q(64) | k(64)]
                # w index = 4g + 2i + par
                qsrc = pa[:, :, :, 0:64].rearrange("p i r c -> p c (i r)")
                ksrc = pa[:, :, :, 64:128].rearrange("p i r c -> p c (i r)")
                nc.vector.tensor_add(
                    qbuf[:, :, 4 * g:4 * g + 4], qsrc,
                    bq4_sb[:].rearrange("p a b c -> p c (a b)"),
                )
                nc.scalar.copy(kbuf[:, :, 4 * g:4 * g + 4], ksrc)

            if prev[0] is not None:
                stage_d(*prev[0])
                prev[0] = None

            # ---- stage C: attention, 4-channel score/exp quads
            obuf = obpool.tile([128, 128, 64], fh)   # [w, h, c]
            fv = vflat[:]
            for c4 in range(0, 64, 4):
                psc = pSC.tile([128, 512], f32)
                for j in range(4):
                    nc.tensor.matmul(
                        psc[:, j * 128:(j + 1) * 128],
                        kbuf[:, c4 + j, :], qbuf[:, c4 + j, :],
                        start=True, stop=True)
                if DEBUG and d == 0 and c4 == 0:
                    scf = ypool.tile([128, 512], f32, tag="dbgs")
                    nc.vector.tensor_copy(scf[:], psc[:])
                    nc.gpsimd.dma_start(dbg_sc[:], scf[:])
                et = etpool.tile([128, 4, 128], fr)
                nc.scalar.activation(
                    et[:], psc[:].rearrange("p (a b) -> p a b", a=4),
                    AF.Exp, bias=eshift_sb[:],
                )
                for jj in (0, 2):
                    c2 = c4 + jj
                    # AV: mm0 -> [0:258] (o_c2 + denom at 128), mm1 ->
                    # [256:512] with rhs starting at channel c2+1's block
                    # (o_c2+1 at 256:384, denom at 384): uniform stride.
                    pav = pAV.tile([128, 512], f32)
                    nc.tensor.matmul(
                        pav[:, 0:258], et[:, jj, :],
                        fv[:, c2 * 129:c2 * 129 + 258],
                        start=True, stop=True)
                    nc.tensor.matmul(
                        pav[:, 256:512], et[:, jj + 1, :],
                        fv[:, (c2 + 1) * 129:(c2 + 1) * 129 + 256],
                        start=True, stop=True)
                    pav2 = pav[:].rearrange("p (a b) -> p a b", a=2)
                    rr = rcpool.tile([128, 2], f32)
                    nc.vector.reciprocal(rr[:], pav2[:, :, 128])
                    # o evac: scale must happen before fp16 (underflow).
                    # Even pairs: direct DVE scale-evac. Odd pairs: stage to
                    # fp32 SBUF on ScalarE + DVE 2x-mode scale.
                    if jj == 0:
                        nc.vector.tensor_scalar_mul(
                            obuf[:, :, c2], pav2[:, 0, 0:128], rr[:, 0:1]
                        )
                        nc.vector.tensor_scalar_mul(
                            obuf[:, :, c2 + 1], pav2[:, 1, 0:128], rr[:, 1:2]
                        )
                    else:
                        ostg = ostgpool.tile([128, 2, 128], f32)
                        nc.scalar.copy(ostg[:], pav2[:, :, 0:128])
                        nc.vector.tensor_scalar_mul(
                            obuf[:, :, c2], ostg[:, 0, :], rr[:, 0:1]
                        )
                        nc.vector.tensor_scalar_mul(
                            obuf[:, :, c2 + 1], ostg[:, 1, :], rr[:, 1:2]
                        )

            if DEBUG and d == 0:
                nc.gpsimd.dma_start(
                    dbg_qk[:, 0:8192], qbuf[:].rearrange("p a b -> p (a b)"))
                nc.gpsimd.dma_start(
                    dbg_qk[:, 8192:16384], kbuf[:].rearrange("p a b -> p (a b)"))
                nc.gpsimd.dma_start(
                    dbg_vb[:], vbuf[:].rearrange("p a b -> p (a b)").bitcast(f32))
                nc.gpsimd.dma_start(
                    dbg_ob[:], obuf[:].rearrange("p a b -> p (a b)"))

            prev[0] = (d, obuf)

        if prev[0] is not None:
            stage_d(*prev[0])
            prev[0] = None

    nc.compile()
    return nc


def _prep_inputs(x, wq, bq, wk, bk, wv, bv, wo, bo):
    """Build per-core input maps."""
    x = np.asarray(x, _f32)[0]           # [64, 48, 128, 128]
    wq2 = np.asarray(wq, _f32) * QSCALE
    bq2 = np.asarray(bq, _f32) * QSCALE
    wk = np.asarray(wk, _f32)
    wv = np.asarray(wv, _f32); bv = np.asarray(bv, _f32)
    wo = np.asarray(wo, _f32); bo = np.asarray(bo, _f32)

    # conv-A weights: [c-pair, 2 x (q|k)]
    A = np.zeros((64, 128), _f32)
    A[:, 0:64] = wq2.T
    A[:, 64:128] = wk.T
    wbda = np.zeros((128, 256), _f32)
    wbda[0:64, 0:128] = A
    wbda[64:128, 128:256] = A
    wbda = round_fp32r(wbda)

    # conv-B weights fp16: blockdiag(wv^T)
    wvbd = np.zeros((128, 128), _f32)
    wvbd[0:64, 0:64] = wv.T
    wvbd[64:128, 64:128] = wv.T
    wvbd = wvbd.astype(_f16)

    bq4 = np.ascontiguousarray(
        np.broadcast_to(bq2[None, None, :], (128, 4, 64)).reshape(128, 256),
        dtype=_f32,
    )

    identh = np.eye(128, dtype=_f32).astype(_f16)

    # blockdiag wo^T (h-parity major): wobd2[64p+c, 64p+c'] = wo[c',c]
    wobd2 = np.zeros((128, 128), _f32)
    woT = wo.T  # [c, c']
    wobd2[0:64, 0:64] = woT
    wobd2[64:128, 64:128] = woT
    wobd2 = wobd2.astype(_f16)

    bo2v = (bo + wo @ bv).astype(_f32)
    bo2i = np.concatenate([bo2v, bo2v]).reshape(128, 1)

    in_maps = []
    for i in range(NCORES):
        xc = x[:, i * DLOC:(i + 1) * DLOC]          # [64, 6, 128, 128]
        xa = np.empty((128, DLOC, 64, 128), _f32)
        xa[0:64] = xc[:, :, 0::2, :]                # w even: [c, d, w2, h]
        xa[64:128] = xc[:, :, 1::2, :]              # w odd
        xa = round_fp32r(xa.reshape(128, DLOC * 64 * 128))
        xbv = np.empty((128, DLOC, 64, 128), _f16)
        xbv[0:64] = xc[:, :, :, 0::2].transpose(0, 1, 3, 2)   # [c, d, h2, w]
        xbv[64:128] = xc[:, :, :, 1::2].transpose(0, 1, 3, 2)
        xb = np.ascontiguousarray(xbv.reshape(128, DLOC * 64 * 128))
        in_maps.append({
            "xa": xa, "xb": xb, "wbda": wbda, "wvbd": wvbd, "bq4": bq4,
            "identh": identh, "wobd2": wobd2, "bo2i": bo2i,
        })
    return in_maps


def _decode_outputs(results):
    """results: per-core dicts with 'y' [DLOC, 4, 128, 2048]."""
    outs = []
    for r in results:
        y = np.asarray(r["y"], _f32).reshape(DLOC, 4, 2, 64, 4, 4, 128)
        # dims: [d, q, p, c', u, t, w]; h = 32q + 8u + 2t + p
        Y = np.transpose(y, (3, 0, 6, 1, 4, 5, 2)).reshape(64, DLOC, 128, 128)
        outs.append(Y)
    return np.concatenate(outs, axis=1)[None]        # [1, 64, 48, 128, 128]


_CACHE = {}


def _get_runner():
    """Build bass program + cached jitted pjrt callable."""
    if "runner" in _CACHE:
        return _CACHE["runner"]
    import jax
    import jax.numpy as jnp
    from jax.sharding import Mesh, PartitionSpec
    from jax.experimental.shard_map import shard_map
    import concourse.mybir as mybir
    from concourse import bass2jax
    from concourse.bass2jax import _bass_exec_p, install_neuronx_cc_hook

    install_neuronx_cc_hook()
    nc = _build_bass()

    partition_name = (
        nc.partition_id_tensor.name if nc.partition_id_tensor else None
    )
    in_names, out_names, out_avals = [], [], []
    for alloc in nc.m.functions[0].allocations:
        if not isinstance(alloc, mybir.MemoryLocationSet):
            continue
        name = alloc.memorylocations[0].name
        if alloc.kind == "ExternalInput":
            if name != partition_name:
                in_names.append(name)
        elif alloc.kind == "ExternalOutput":
            out_names.append(name)
            out_avals.append(
                jax.core.ShapedArray(
                    tuple(alloc.tensor_shape), mybir.dt.np(alloc.dtype)
                )
            )
    n_params = len(in_names)
    zero_shapes = [(a.shape, a.dtype) for a in out_avals]
    all_in_names = list(in_names) + list(out_names)
    if partition_name is not None:
        all_in_names.append(partition_name)

    def _body(*args):
        operands = list(args)
        if partition_name is not None:
            operands.append(bass2jax.partition_id_tensor())
        outs = _bass_exec_p.bind(
            *operands,
            out_avals=tuple(out_avals),
            in_names=tuple(all_in_names),
            out_names=tuple(out_names),
            lowering_input_output_aliases=(),
            sim_require_finite=True,
            sim_require_nnan=True,
            nc=nc,
        )
        return tuple(outs)

    devices = jax.devices()[:NCORES]
    mesh = Mesh(np.asarray(devices), ("core",))
    n_outs = len(out_names)
    in_specs = (PartitionSpec("core"),) * (n_params + n_outs)
    out_specs = (PartitionSpec("core"),) * n_outs
    donate = tuple(range(n_params, n_params + n_outs))
    sharded = jax.jit(
        shard_map(_body, mesh=mesh, in_specs=in_specs, out_specs=out_specs,
                  check_rep=False),
        donate_argnums=donate,
        keep_unused=True,
    )

    def run(in_maps):
        concat_in = [
            np.concatenate([np.asarray(in_maps[c][nm]) for c in range(NCORES)],
                           axis=0)
            for nm in in_names
        ]
        concat_zeros = [
            np.zeros((NCORES * s[0],) + tuple(s[1:]), dt)
            for (s, dt) in zero_shapes
        ]
        out = sharded(*concat_in, *concat_zeros)
        res = []
        for c in range(NCORES):
            res.append({
                nm: np.asarray(out[i]).reshape(NCORES, *zero_shapes[i][0])[c]
                for i, nm in enumerate(out_names)
            })
        return res, (sharded, in_names, zero_shapes, out_names)

    _CACHE["runner"] = run
    return run


def kernel(**inputs):
    run = _get_runner()
    in_maps = _prep_inputs(**inputs)
    results, _ = run(in_maps)
    return _decode_outputs(results)


if __name__ == "__main__":
    import reference
    t0 = time.time()
    ins = {k: np.asarray(v) for k, v in reference.setup_inputs().items()}
    exp = np.asarray(reference.reference(**ins))
    t1 = time.time()
    print(f"reference: {t1 - t0:.1f}s", flush=True)
    act = kernel(**ins)
    t2 = time.time()
    print(f"kernel: {t2 - t1:.1f}s", flush=True)
    err = np.abs(act - exp)
    scale = np.abs(exp).mean()
    print(f"abs err max={err.max():.3e} mean={err.mean():.3e} "
          f"rel(max/scale)={err.max() / scale:.3e} "
          f"rel_mean={(err / (np.abs(exp) + 1e-6)).mean():.3e}", flush=True)


# revision 51
# speedup vs baseline: 2.2052x; 1.0558x over previous
"""Trainium2 Bass kernel for nn_Attention_3d (B=1, C=64, D=48, W=128, H=128, 8 heads).

Sharding: depth D split across 8 cores (6 slices each), weights replicated.

v2 design (evacuation-balanced; cost-model ~322us/core vs 711us baseline):
  - conv-A (Q,K only): stationary = x w-parity pair chunks [c-pair(128) x h(128)]
    fp32r, moving = blockdiag([wq'|wk]^T) fp32r [128,256] -> PSUM per wp.
    Evac: Q via DVE tensor_add(+bq') -> qbuf fp16; K via ScalarE copy -> kbuf
    fp16.  (bk is dropped entirely: it only shifts scores by a per-query
    constant, which softmax cancels; bv is folded into the output bias.)
  - conv-B (V, produced pre-transposed): x is ALSO loaded in an h-parity
    layout (fp16), stationary = [c-pair(h-par) x w(128)], moving =
    blockdiag(wv^T) fp16 [128,128] -> PSUM [w, (h-par, c)] per h-pair ->
    evac straight into v_nat [w, c, h+ones] fp32r.  This kills the separate
    V-natural evac + PE transposes of the original design.
  - scores: fp16 matmuls at N=128 (no fp32r>=256 constraint), 4 channels
    per PSUM bank, one FD=512 exp(x-80) on ScalarE -> et fp32r (the -80
    shift keeps exp finite; denominators renormalize it away).
  - AV (fp32r): both channels of a pair in ONE psum bank: mm0 -> [0:258]
    (o|denom|garbage), mm1 -> [256:512] with rhs starting at channel c+1's
    v-block, so useful cols sit at uniform stride; rhs = flat v_nat with
    interleaved ones columns.  reciprocal(denoms) on DVE; o evac scales
    during PSUM evac (values underflow fp16 unscaled): half direct DVE
    tensor_scalar, half ScalarE-staged fp32 + DVE 2x-mode tensor_scalar
    -> obuf fp16 [w, h, c].
  - out: PE-transpose obuf fp16 blocks [w, (2h,c)] -> [(hp,c), w] psum fp16
    (1 cyc/row), evac fp16->fp16 (DVE 2x mode) -> och, final conv fp16
    blockdiag(wo^T) N=512, bias bo' = bo + wo@bv on ScalarE evac, y DMA
    per 2048 cols.  Stage D of slice d is emitted after conv stages of d+1.
  - Pipelining: qbuf/kbuf/obuf double-buffered (conv-A/Q/K evacs of d+1
    overlap attention of d); input DMAs chunked 4x; const loads on the SP
    DMA queue; PSUM: pA2/pCB1/pSC1/pAV2/pOT1/pPY1 banks.  Temporally idle
    pools are borrowed to deepen chains: conv-B ping-pongs pCB with pSC
    (scores pool is idle in the inter-attention window), d=0 conv-A borrows
    pAV, and the final slice's output stage borrows pA/pSC.
"""
import time
import numpy as np
import ml_dtypes
from contextlib import ExitStack

B, C, D, W, H = 1, 64, 48, 128, 128
NCORES = 8
DLOC = D // NCORES  # 6
QSCALE = float(8.0 ** -0.25)

_f32 = np.float32
_f16 = np.float16
DEBUG = False


def round_fp32r(x):
    u = np.ascontiguousarray(x, dtype=np.float32).view(np.uint32)
    lsb = (u >> 12) & 1
    u = (u + 0x7FF + lsb) & np.uint32(0xFFFFF000)
    return u.view(np.float32)


def _build_bass():
    import concourse.bacc as bacc
    import concourse.bass as bass
    import concourse.mybir as mybir
    import concourse.tile as tile

    fr = mybir.dt.float32r
    f32 = mybir.dt.float32
    fh = mybir.dt.float16
    AF = mybir.ActivationFunctionType

    nc = bacc.Bacc("TRN2", target_bir_lowering=False, debug=False)

    xa_d = nc.dram_tensor("xa", [128, DLOC * 64 * 128], fr, kind="ExternalInput").ap()
    xb_d = nc.dram_tensor("xb", [128, DLOC * 64 * 128], fh, kind="ExternalInput").ap()
    wbda_d = nc.dram_tensor("wbda", [128, 256], fr, kind="ExternalInput").ap()
    wvbd_d = nc.dram_tensor("wvbd", [128, 128], fh, kind="ExternalInput").ap()
    bq4_d = nc.dram_tensor("bq4", [128, 256], f32, kind="ExternalInput").ap()
    identh_d = nc.dram_tensor("identh", [128, 128], fh, kind="ExternalInput").ap()
    wobd2_d = nc.dram_tensor("wobd2", [128, 128], fh, kind="ExternalInput").ap()
    bo2i_d = nc.dram_tensor("bo2i", [128, 1], f32, kind="ExternalInput").ap()
    y_d = nc.dram_tensor("y", [DLOC, 4, 128, 2048], f32, kind="ExternalOutput").ap()
    if DEBUG:
        dbg_qk = nc.dram_tensor("dbg_qk", [128, 2 * 64 * 128], fh, kind="ExternalOutput").ap()
        dbg_vb = nc.dram_tensor("dbg_vb", [128, 64 * 129], f32, kind="ExternalOutput").ap()
        dbg_ob = nc.dram_tensor("dbg_ob", [128, 64 * 128], fh, kind="ExternalOutput").ap()
        dbg_sc = nc.dram_tensor("dbg_sc", [128, 512], f32, kind="ExternalOutput").ap()

    with tile.TileContext(nc) as tc, ExitStack() as ctx:
        const = ctx.enter_context(tc.tile_pool(name="const", bufs=1))
        xapool = ctx.enter_context(tc.tile_pool(name="xapool", bufs=1))
        xbpool = ctx.enter_context(tc.tile_pool(name="xbpool", bufs=1))
        qkpool = ctx.enter_context(tc.tile_pool(name="qkpool", bufs=2))
        vbpool = ctx.enter_context(tc.tile_pool(name="vbpool", bufs=1))
        etpool = ctx.enter_context(tc.tile_pool(name="etpool", bufs=2))
        rcpool = ctx.enter_context(tc.tile_pool(name="rcpool", bufs=6))
        obpool = ctx.enter_context(tc.tile_pool(name="obpool", bufs=2))
        ochpool = ctx.enter_context(tc.tile_pool(name="ochpool", bufs=2))
        ypool = ctx.enter_context(tc.tile_pool(name="ypool", bufs=2))
        ostgpool = ctx.enter_context(tc.tile_pool(name="ostgpool", bufs=3))

        # PSUM: 8 banks total (bufs are per-tag/pool)
        pA = ctx.enter_context(tc.tile_pool(name="pA", bufs=2, space="PSUM"))
        pCB = ctx.enter_context(tc.tile_pool(name="pCB", bufs=1, space="PSUM"))
        pSC = ctx.enter_context(tc.tile_pool(name="pSC", bufs=1, space="PSUM"))
        pAV = ctx.enter_context(tc.tile_pool(name="pAV", bufs=2, space="PSUM"))
        pOT = ctx.enter_context(tc.tile_pool(name="pOT", bufs=1, space="PSUM"))
        pPY = ctx.enter_context(tc.tile_pool(name="pPY", bufs=1, space="PSUM"))

        # constants: memsets on DVE, const loads on the idle SP DMA queue
        eshift_sb = const.tile([128, 1], f32)
        nc.vector.memset(eshift_sb[:], -80.0)
        ones_f = const.tile([128, 64], f32)
        nc.vector.memset(ones_f[:], 1.0)
        wvbd_sb = const.tile([128, 128], fh)
        nc.sync.dma_start(wvbd_sb[:], wvbd_d[:])
        wbda_sb = const.tile([128, 256], fr)
        nc.sync.dma_start(wbda_sb[:], wbda_d[:])
        bq4_sb = const.tile([128, 2, 2, 64], f32)
        nc.sync.dma_start(
            bq4_sb[:], bq4_d[:].rearrange("p (a b c) -> p a b c", a=2, b=2)
        )
        identh_sb = const.tile([128, 128], fh)
        nc.sync.dma_start(identh_sb[:], identh_d[:])
        wobd2_sb = const.tile([128, 128], fh)
        nc.sync.dma_start(wobd2_sb[:], wobd2_d[:])
        bo2i_sb = const.tile([128, 1], f32)
        nc.sync.dma_start(bo2i_sb[:], bo2i_d[:])

        def stage_d(d, obuf, last=False):
            # ---- stage D: O transpose + final conv, 8 groups of 8 h-pairs.
            # For the final slice, borrow the idle conv pools to deepen the
            # PSUM pipeline (nothing else needs them then).
            for q8 in range(8):
                if q8 % 2 == 0:
                    yb = ypool.tile([128, 4, 512], f32, name="yb")
                if last and q8 % 2 == 1:
                    pt = pSC.tile([128, 1024], fh, tag="psc", name="pt")
                else:
                    pt = pOT.tile([128, 1024], fh, name="pt")
                for t in range(8):
                    hp = 8 * q8 + t
                    nc.tensor.transpose(
                        pt[:, t * 128:(t + 1) * 128],
                        obuf[:, 2 * hp:2 * hp + 2, :].rearrange(
                            "p a c -> p (a c)"),
                        identh_sb[:],
                    )
                och = ochpool.tile([128, 1024], fh, name="och")
                nc.vector.tensor_copy(och[:], pt[:])
                for half in range(2):
                    if last and (2 * q8 + half) % 2 == 1:
                        py = pA.tile([128, 512], f32, tag="pat", name="py")
                    else:
                        py = pPY.tile([128, 512], f32, name="py")
                    nc.tensor.matmul(
                        py[:], wobd2_sb[:], och[:, half * 512:(half + 1) * 512],
                        start=True, stop=True)
                    nc.scalar.activation(
                        yb[:, 2 * (q8 % 2) + half, :], py[:], AF.Identity,
                        bias=bo2i_sb[:])
                if q8 % 2 == 1:
                    nc.gpsimd.dma_start(
                        y_d[d, q8 // 2], yb[:].rearrange("p a b -> p (a b)")
                    )

        prev = [None]

        for d in range(DLOC):
            # ---- input loads
            xa_sb = xapool.tile([128, 64 * 128], fr)
            xb_sb = xbpool.tile([128, 64 * 128], fh)
            for ch in range(4):
                lo, hi = ch * 2048, (ch + 1) * 2048
                nc.gpsimd.dma_start(xb_sb[:, lo:hi], xb_d[:, d * 8192 + lo:d * 8192 + hi])
                nc.gpsimd.dma_start(xa_sb[:, lo:hi], xa_d[:, d * 8192 + lo:d * 8192 + hi])

            # ---- stage A: conv-B -> v_nat [w, c, h] fp32r (+ ones col 128)
            # flat [128, 64*129 + 126] with tail pad so the last AV mm's
            # 256-wide moving slice stays in bounds.
            vflat = vbpool.tile([128, 64 * 129 + 128], fr)
            vbuf = vflat[:, 0:64 * 129].rearrange("p (c h) -> p c h", c=64)
            nc.gpsimd.memset(vflat[:, 64 * 129:].bitcast(f32), 0.0)
            nc.vector.tensor_copy(vbuf[:, :, 128], ones_f[:])
            for g in range(16):
                if g % 2 == 1:
                    pbt = pSC.tile([128, 512], f32, tag="psc", name="pbt")
                else:
                    pbt = pCB.tile([128, 512], f32, tag="pbt", name="pbt")
                pb = pbt[:].rearrange("p (a b) -> p a b", a=4)
                for t in range(4):
                    hp = 4 * g + t
                    nc.tensor.matmul(
                        pb[:, t, :],
                        xb_sb[:, hp * 128:(hp + 1) * 128],
                        wvbd_sb[:],
                        start=True, stop=True,
                    )
                # src [w, t(4), par(2), c(64)] -> dst vbuf[w, c, 8h]
                src = pb[:].rearrange("p t (r c) -> p c (t r)", r=2)
                if g % 2 == 0:
                    nc.vector.tensor_copy(vbuf[:, :, 8 * g:8 * g + 8], src)
                else:
                    nc.scalar.copy(vbuf[:, :, 8 * g:8 * g + 8], src)

            # ---- stage B: conv-A -> qbuf, kbuf fp16 [h, c, w]
            qbuf = qkpool.tile([128, 64, 128], fh, tag="q")
            kbuf = qkpool.tile([128, 64, 128], fh, tag="k")
            for g in range(32):
                if d == 0 and g % 3 == 2:
                    pat = pAV.tile([128, 512], f32, tag="pav", name="pat")
                else:
                    pat = pA.tile([128, 512], f32, tag="pat", name="pat")
                # [h, wp(2), par(2), qk*64]
                pa = pat[:].rearrange("p (a b c) -> p a b c", a=2, b=2)
                for i in range(2):
                    wp = 2 * g + i
                    nc.tensor.matmul(
                        pa[:, i, :, :].rearrange("p a b -> p (a b)"),
                        xa_sb[:, wp * 128:(wp + 1) * 128],
                        wbda_sb[:],
                        start=True, stop=True,
                    )
                # cols per parity: [q(64) | k(64)]
                # w index = 4g + 2i + par
                qsrc = pa[:, :, :, 0:64].rearrange("p i r c -> p c (i r)")
                ksrc = pa[:, :, :, 64:128].rearrange("p i r c -> p c (i r)")
                nc.vector.tensor_add(
                    qbuf[:, :, 4 * g:4 * g + 4], qsrc,
                    bq4_sb[:].rearrange("p a b c -> p c (a b)"),
                )
                nc.scalar.copy(kbuf[:, :, 4 * g:4 * g + 4], ksrc)

            if prev[0] is not None:
                stage_d(*prev[0])
                prev[0] = None

            # ---- stage C: attention, 4-channel score/exp quads
            obuf = obpool.tile([128, 128, 64], fh)   # [w, h, c]
            fv = vflat[:]
            for c4 in range(0, 64, 4):
                psc = pSC.tile([128, 512], f32)
                for j in range(4):
                    nc.tensor.matmul(
                        psc[:, j * 128:(j + 1) * 128],
                        kbuf[:, c4 + j, :], qbuf[:, c4 + j, :],
                        start=True, stop=True)
                if DEBUG and d == 0 and c4 == 0:
                    scf = ypool.tile([128, 512], f32, tag="dbgs")
                    nc.vector.tensor_copy(scf[:], psc[:])
                    nc.gpsimd.dma_start(dbg_sc[:], scf[:])
                et = etpool.tile([128, 4, 128], fr)
                nc.scalar.activation(
                    et[:], psc[:].rearrange("p (a b) -> p a b", a=4),
                    AF.Exp, bias=eshift_sb[:],
                )
                for jj in (0, 2):
                    c2 = c4 + jj
                    # AV: mm0 -> [0:258] (o_c2 + denom at 128), mm1 ->
                    # [256:512] with rhs starting at channel c2+1's block
                    # (o_c2+1 at 256:384, denom at 384): uniform stride.
                    pav = pAV.tile([128, 512], f32)
                    nc.tensor.matmul(
                        pav[:, 0:258], et[:, jj, :],
                        fv[:, c2 * 129:c2 * 129 + 258],
                        start=True, stop=True)
                    nc.tensor.matmul(
                        pav[:, 256:512], et[:, jj + 1, :],
                        fv[:, (c2 + 1) * 129:(c2 + 1) * 129 + 256],
                        start=True, stop=True)
                    pav2 = pav[:].rearrange("p (a b) -> p a b", a=2)
                    rr = rcpool.tile([128, 2], f32)
                    nc.vector.reciprocal(rr[:], pav2[:, :, 128])
                    # o evac: scale must happen before fp16 (underflow).
                    # Even pairs: direct DVE scale-evac. Odd pairs: stage to
                    # fp32 SBUF on ScalarE + DVE 2x-mode scale.
                    if jj == 0:
                        nc.vector.tensor_scalar_mul(
                            obuf[:, :, c2], pav2[:, 0, 0:128], rr[:, 0:1]
                        )
                        nc.vector.tensor_scalar_mul(
                            obuf[:, :, c2 + 1], pav2[:, 1, 0:128], rr[:, 1:2]
                        )
                    else:
                        ostg = ostgpool.tile([128, 2, 128], f32)
                        nc.scalar.copy(ostg[:], pav2[:, :, 0:128])
                        nc.vector.tensor_scalar_mul(
                            obuf[:, :, c2], ostg[:, 0, :], rr[:, 0:1]
                        )
                        nc.vector.tensor_scalar_mul(
                            obuf[:, :, c2 + 1], ostg[:, 1, :], rr[:, 1:2]
                        )

            if DEBUG and d == 0:
                nc.gpsimd.dma_start(
                    dbg_qk[:, 0:8192], qbuf[:].rearrange("p a b -> p (a b)"))
                nc.gpsimd.dma_start(
                    dbg_qk[:, 8192:16384], kbuf[:].rearrange("p a b -> p (a b)"))
                nc.gpsimd.dma_start(
                    dbg_vb[:], vbuf[:].rearrange("p a b -> p (a b)").bitcast(f32))
                nc.gpsimd.dma_start(
                    dbg_ob[:], obuf[:].rearrange("p a b -> p (a b)"))

            prev[0] = (d, obuf)

        if prev[0] is not None:
            stage_d(*prev[0], last=True)
            prev[0] = None

    nc.compile()
    return nc


def _prep_inputs(x, wq, bq, wk, bk, wv, bv, wo, bo):
    """Build per-core input maps."""
    x = np.asarray(x, _f32)[0]           # [64, 48, 128, 128]
    wq2 = np.asarray(wq, _f32) * QSCALE
    bq2 = np.asarray(bq, _f32) * QSCALE
    wk = np.asarray(wk, _f32)
    wv = np.asarray(wv, _f32); bv = np.asarray(bv, _f32)
    wo = np.asarray(wo, _f32); bo = np.asarray(bo, _f32)

    # conv-A weights: [c-pair, 2 x (q|k)]
    A = np.zeros((64, 128), _f32)
    A[:, 0:64] = wq2.T
    A[:, 64:128] = wk.T
    wbda = np.zeros((128, 256), _f32)
    wbda[0:64, 0:128] = A
    wbda[64:128, 128:256] = A
    wbda = round_fp32r(wbda)

    # conv-B weights fp16: blockdiag(wv^T)
    wvbd = np.zeros((128, 128), _f32)
    wvbd[0:64, 0:64] = wv.T
    wvbd[64:128, 64:128] = wv.T
    wvbd = wvbd.astype(_f16)

    bq4 = np.ascontiguousarray(
        np.broadcast_to(bq2[None, None, :], (128, 4, 64)).reshape(128, 256),
        dtype=_f32,
    )

    identh = np.eye(128, dtype=_f32).astype(_f16)

    # blockdiag wo^T (h-parity major): wobd2[64p+c, 64p+c'] = wo[c',c]
    wobd2 = np.zeros((128, 128), _f32)
    woT = wo.T  # [c, c']
    wobd2[0:64, 0:64] = woT
    wobd2[64:128, 64:128] = woT
    wobd2 = wobd2.astype(_f16)

    bo2v = (bo + wo @ bv).astype(_f32)
    bo2i = np.concatenate([bo2v, bo2v]).reshape(128, 1)

    in_maps = []
    for i in range(NCORES):
        xc = x[:, i * DLOC:(i + 1) * DLOC]          # [64, 6, 128, 128]
        xa = np.empty((128, DLOC, 64, 128), _f32)
        xa[0:64] = xc[:, :, 0::2, :]                # w even: [c, d, w2, h]
        xa[64:128] = xc[:, :, 1::2, :]              # w odd
        xa = round_fp32r(xa.reshape(128, DLOC * 64 * 128))
        xbv = np.empty((128, DLOC, 64, 128), _f16)
        xbv[0:64] = xc[:, :, :, 0::2].transpose(0, 1, 3, 2)   # [c, d, h2, w]
        xbv[64:128] = xc[:, :, :, 1::2].transpose(0, 1, 3, 2)
        xb = np.ascontiguousarray(xbv.reshape(128, DLOC * 64 * 128))
        in_maps.append({
            "xa": xa, "xb": xb, "wbda": wbda, "wvbd": wvbd, "bq4": bq4,
            "identh": identh, "wobd2": wobd2, "bo2i": bo2i,
        })
    return in_maps


def _decode_outputs(results):
    """results: per-core dicts with 'y' [DLOC, 4, 128, 2048]."""
    outs = []
    for r in results:
        y = np.asarray(r["y"], _f32).reshape(DLOC, 4, 2, 64, 4, 4, 128)
        # dims: [d, q, p, c', u, t, w]; h = 32q + 8u + 2t + p
        Y = np.transpose(y, (3, 0, 6, 1, 4, 5, 2)).reshape(64, DLOC, 128, 128)
        outs.append(Y)
    return np.concatenate(outs, axis=1)[None]        # [1, 64, 48, 128, 128]


_CACHE = {}


def _get_runner():
    """Build bass program + cached jitted pjrt callable."""
    if "runner" in _CACHE:
        return _CACHE["runner"]
    import jax
    import jax.numpy as jnp
    from jax.sharding import Mesh, PartitionSpec
    from jax.experimental.shard_map import shard_map
    import concourse.mybir as mybir
    from concourse import bass2jax
    from concourse.bass2jax import _bass_exec_p, install_neuronx_cc_hook

    install_neuronx_cc_hook()
    nc = _build_bass()

    partition_name = (
        nc.partition_id_tensor.name if nc.partition_id_tensor else None
    )
    in_names, out_names, out_avals = [], [], []
    for alloc in nc.m.functions[0].allocations:
        if not isinstance(alloc, mybir.MemoryLocationSet):
            continue
        name = alloc.memorylocations[0].name
        if alloc.kind == "ExternalInput":
            if name != partition_name:
                in_names.append(name)
        elif alloc.kind == "ExternalOutput":
            out_names.append(name)
            out_avals.append(
                jax.core.ShapedArray(
                    tuple(alloc.tensor_shape), mybir.dt.np(alloc.dtype)
                )
            )
    n_params = len(in_names)
    zero_shapes = [(a.shape, a.dtype) for a in out_avals]
    all_in_names = list(in_names) + list(out_names)
    if partition_name is not None:
        all_in_names.append(partition_name)

    def _body(*args):
        operands = list(args)
        if partition_name is not None:
            operands.append(bass2jax.partition_id_tensor())
        outs = _bass_exec_p.bind(
            *operands,
            out_avals=tuple(out_avals),
            in_names=tuple(all_in_names),
            out_names=tuple(out_names),
            lowering_input_output_aliases=(),
            sim_require_finite=True,
            sim_require_nnan=True,
            nc=nc,
        )
        return tuple(outs)

    devices = jax.devices()[:NCORES]
    mesh = Mesh(np.asarray(devices), ("core",))
    n_outs = len(out_names)
    in_specs = (PartitionSpec("core"),) * (n_params + n_outs)
    out_specs = (PartitionSpec("core"),) * n_outs
    donate = tuple(range(n_params, n_params + n_outs))
    sharded = jax.jit(
        shard_map(_body, mesh=mesh, in_specs=in_specs, out_specs=out_specs,
                  check_rep=False),
        donate_argnums=donate,
        keep_unused=True,
    )

    def run(in_maps):
        concat_in = [
            np.concatenate([np.asarray(in_maps[c][nm]) for c in range(NCORES)],
                           axis=0)
            for nm in in_names
        ]
        concat_zeros = [
            np.zeros((NCORES * s[0],) + tuple(s[1:]), dt)
            for (s, dt) in zero_shapes
        ]
        out = sharded(*concat_in, *concat_zeros)
        res = []
        for c in range(NCORES):
            res.append({
                nm: np.asarray(out[i]).reshape(NCORES, *zero_shapes[i][0])[c]
                for i, nm in enumerate(out_names)
            })
        return res, (sharded, in_names, zero_shapes, out_names)

    _CACHE["runner"] = run
    return run


def kernel(**inputs):
    run = _get_runner()
    in_maps = _prep_inputs(**inputs)
    results, _ = run(in_maps)
    return _decode_outputs(results)


if __name__ == "__main__":
    import reference
    t0 = time.time()
    ins = {k: np.asarray(v) for k, v in reference.setup_inputs().items()}
    exp = np.asarray(reference.reference(**ins))
    t1 = time.time()
    print(f"reference: {t1 - t0:.1f}s", flush=True)
    act = kernel(**ins)
    t2 = time.time()
    print(f"kernel: {t2 - t1:.1f}s", flush=True)
    err = np.abs(act - exp)
    scale = np.abs(exp).mean()
    print(f"abs err max={err.max():.3e} mean={err.mean():.3e} "
          f"rel(max/scale)={err.max() / scale:.3e} "
          f"rel_mean={(err / (np.abs(exp) + 1e-6)).mean():.3e}", flush=True)


# revision 62
# speedup vs baseline: 2.2312x; 1.0118x over previous
"""Trainium2 Bass kernel for nn_Attention_3d (B=1, C=64, D=48, W=128, H=128, 8 heads).

Sharding: depth D split across 8 cores (6 slices each), weights replicated.

v2 design (evacuation-balanced; cost-model ~318us/core vs 711us baseline):
  - conv-A (Q,K only): stationary = x w-parity pair chunks [c-pair(128) x h(128)]
    fp32r, moving = blockdiag([wq'|wk]^T) fp32r [128,256] -> PSUM per wp.
    Evac: Q via DVE tensor_add(+bq') -> qbuf fp16; K via ScalarE copy -> kbuf
    fp16.  (bk is dropped entirely: it only shifts scores by a per-query
    constant, which softmax cancels; bv is folded into the output bias.)
  - conv-B (V, produced pre-transposed): x is ALSO loaded in an h-parity
    layout (fp16), stationary = [c-pair(h-par) x w(128)], moving =
    blockdiag(wv^T) fp16 [128,128] -> PSUM [w, (h-par, c)] per h-pair ->
    evac straight into v_nat [w, c, h+ones] fp32r.  This kills the separate
    V-natural evac + PE transposes of the original design.
  - scores: fp16 matmuls at N=128 (no fp32r>=256 constraint), 4 channels
    per PSUM bank, one FD=512 exp(x-80) on ScalarE -> et fp32r (the -80
    shift keeps exp finite; denominators renormalize it away).
  - AV (fp32r): both channels of a pair in ONE psum bank: mm0 -> [0:258]
    (o|denom|garbage), mm1 -> [256:512] with rhs starting at channel c+1's
    v-block, so useful cols sit at uniform stride; rhs = flat v_nat with
    interleaved ones columns.  reciprocal(denoms) on DVE; o evac scales
    during PSUM evac (values underflow fp16 unscaled): half direct DVE
    tensor_scalar, half ScalarE-staged fp32 + DVE 2x-mode tensor_scalar
    -> obuf fp16 [w, h, c].
  - out: PE-transpose obuf fp16 blocks [w, (2h,c)] -> [(hp,c), w] psum fp16
    (1 cyc/row), evac fp16->fp16 (DVE 2x mode) -> och, final conv fp16
    blockdiag(wo^T) N=512, bias bo' = bo + wo@bv on ScalarE evac, y DMA
    per 2048 cols.  Stage D of slice d is emitted after conv stages of d+1.
  - Pipelining: qbuf/kbuf/obuf double-buffered (conv-A/Q/K evacs of d+1
    overlap attention of d); input DMAs chunked 4x; const loads on the SP
    DMA queue; PSUM: pA2/pCB1/pSC1/pAV2/pOT1/pPY1 banks.  Temporally idle
    pools are borrowed to deepen chains: conv-B ping-pongs pCB with pSC
    (scores pool is idle in the inter-attention window), d=0 conv-A borrows
    pAV, and the final slice's output stage borrows pA/pSC.
"""
import time
import numpy as np
import ml_dtypes
from contextlib import ExitStack

B, C, D, W, H = 1, 64, 48, 128, 128
NCORES = 8
DLOC = D // NCORES  # 6
QSCALE = float(8.0 ** -0.25)

_f32 = np.float32
_f16 = np.float16
DEBUG = False


def round_fp32r(x):
    u = np.ascontiguousarray(x, dtype=np.float32).view(np.uint32)
    lsb = (u >> 12) & 1
    u = (u + 0x7FF + lsb) & np.uint32(0xFFFFF000)
    return u.view(np.float32)


def _build_bass():
    import concourse.bacc as bacc
    import concourse.bass as bass
    import concourse.mybir as mybir
    import concourse.tile as tile

    fr = mybir.dt.float32r
    f32 = mybir.dt.float32
    fh = mybir.dt.float16
    AF = mybir.ActivationFunctionType

    nc = bacc.Bacc("TRN2", target_bir_lowering=False, debug=False)

    xa_d = nc.dram_tensor("xa", [128, DLOC * 64 * 128], fr, kind="ExternalInput").ap()
    xb_d = nc.dram_tensor("xb", [128, DLOC * 64 * 128], fh, kind="ExternalInput").ap()
    wbda_d = nc.dram_tensor("wbda", [128, 256], fr, kind="ExternalInput").ap()
    wvbd_d = nc.dram_tensor("wvbd", [128, 128], fh, kind="ExternalInput").ap()
    bq4_d = nc.dram_tensor("bq4", [128, 256], f32, kind="ExternalInput").ap()
    identh_d = nc.dram_tensor("identh", [128, 128], fh, kind="ExternalInput").ap()
    wobd2_d = nc.dram_tensor("wobd2", [128, 128], fh, kind="ExternalInput").ap()
    bo2i_d = nc.dram_tensor("bo2i", [128, 1], f32, kind="ExternalInput").ap()
    y_d = nc.dram_tensor("y", [DLOC, 4, 128, 2048], f32, kind="ExternalOutput").ap()
    if DEBUG:
        dbg_qk = nc.dram_tensor("dbg_qk", [128, 2 * 64 * 128], fh, kind="ExternalOutput").ap()
        dbg_vb = nc.dram_tensor("dbg_vb", [128, 64 * 129], f32, kind="ExternalOutput").ap()
        dbg_ob = nc.dram_tensor("dbg_ob", [128, 64 * 128], fh, kind="ExternalOutput").ap()
        dbg_sc = nc.dram_tensor("dbg_sc", [128, 512], f32, kind="ExternalOutput").ap()

    with tile.TileContext(nc) as tc, ExitStack() as ctx:
        const = ctx.enter_context(tc.tile_pool(name="const", bufs=1))
        xapool = ctx.enter_context(tc.tile_pool(name="xapool", bufs=1))
        xbpool = ctx.enter_context(tc.tile_pool(name="xbpool", bufs=1))
        qkpool = ctx.enter_context(tc.tile_pool(name="qkpool", bufs=2))
        vbpool = ctx.enter_context(tc.tile_pool(name="vbpool", bufs=1))
        etpool = ctx.enter_context(tc.tile_pool(name="etpool", bufs=2))
        rcpool = ctx.enter_context(tc.tile_pool(name="rcpool", bufs=6))
        obpool = ctx.enter_context(tc.tile_pool(name="obpool", bufs=2))
        ochpool = ctx.enter_context(tc.tile_pool(name="ochpool", bufs=3))
        ypool = ctx.enter_context(tc.tile_pool(name="ypool", bufs=2))
        ostgpool = ctx.enter_context(tc.tile_pool(name="ostgpool", bufs=2))

        # PSUM: 8 banks total (bufs are per-tag/pool)
        pA = ctx.enter_context(tc.tile_pool(name="pA", bufs=2, space="PSUM"))
        pCB = ctx.enter_context(tc.tile_pool(name="pCB", bufs=1, space="PSUM"))
        pSC = ctx.enter_context(tc.tile_pool(name="pSC", bufs=1, space="PSUM"))
        pAV = ctx.enter_context(tc.tile_pool(name="pAV", bufs=2, space="PSUM"))
        pOT = ctx.enter_context(tc.tile_pool(name="pOT", bufs=1, space="PSUM"))
        pPY = ctx.enter_context(tc.tile_pool(name="pPY", bufs=1, space="PSUM"))

        # constants: memsets on DVE, const loads on the idle SP DMA queue
        eshift_sb = const.tile([128, 1], f32)
        nc.vector.memset(eshift_sb[:], -80.0)
        ones_f = const.tile([128, 64], f32)
        nc.vector.memset(ones_f[:], 1.0)
        wvbd_sb = const.tile([128, 128], fh)
        nc.sync.dma_start(wvbd_sb[:], wvbd_d[:])
        wbda_sb = const.tile([128, 256], fr)
        nc.sync.dma_start(wbda_sb[:], wbda_d[:])
        bq4_sb = const.tile([128, 2, 2, 64], f32)
        nc.sync.dma_start(
            bq4_sb[:], bq4_d[:].rearrange("p (a b c) -> p a b c", a=2, b=2)
        )
        identh_sb = const.tile([128, 128], fh)
        nc.sync.dma_start(identh_sb[:], identh_d[:])
        wobd2_sb = const.tile([128, 128], fh)
        nc.sync.dma_start(wobd2_sb[:], wobd2_d[:])
        bo2i_sb = const.tile([128, 1], f32)
        nc.sync.dma_start(bo2i_sb[:], bo2i_d[:])

        def stage_d(d, obuf, last=False):
            # ---- stage D: O transpose + final conv, 8 groups of 8 h-pairs.
            # For the final slice, borrow the idle conv pools to deepen the
            # PSUM pipeline (nothing else needs them then).
            for q8 in range(8):
                if q8 % 2 == 0:
                    yb = ypool.tile([128, 4, 512], f32, name="yb")
                if last and q8 % 2 == 1:
                    pt = pSC.tile([128, 1024], fh, tag="psc", name="pt")
                else:
                    pt = pOT.tile([128, 1024], fh, name="pt")
                for t in range(8):
                    hp = 8 * q8 + t
                    nc.tensor.transpose(
                        pt[:, t * 128:(t + 1) * 128],
                        obuf[:, 2 * hp:2 * hp + 2, :].rearrange(
                            "p a c -> p (a c)"),
                        identh_sb[:],
                    )
                och = ochpool.tile([128, 1024], fh, name="och")
                nc.vector.tensor_copy(och[:], pt[:])
                for half in range(2):
                    if last and (2 * q8 + half) % 2 == 1:
                        py = pA.tile([128, 512], f32, tag="pat", name="py")
                    else:
                        py = pPY.tile([128, 512], f32, name="py")
                    nc.tensor.matmul(
                        py[:], wobd2_sb[:], och[:, half * 512:(half + 1) * 512],
                        start=True, stop=True)
                    nc.scalar.activation(
                        yb[:, 2 * (q8 % 2) + half, :], py[:], AF.Identity,
                        bias=bo2i_sb[:])
                if q8 % 2 == 1:
                    nc.gpsimd.dma_start(
                        y_d[d, q8 // 2], yb[:].rearrange("p a b -> p (a b)")
                    )

        prev = [None]

        for d in range(DLOC):
            # ---- input loads
            xa_sb = xapool.tile([128, 64 * 128], fr)
            xb_sb = xbpool.tile([128, 64 * 128], fh)
            for ch in range(4):
                lo, hi = ch * 2048, (ch + 1) * 2048
                nc.gpsimd.dma_start(xb_sb[:, lo:hi], xb_d[:, d * 8192 + lo:d * 8192 + hi])
                nc.gpsimd.dma_start(xa_sb[:, lo:hi], xa_d[:, d * 8192 + lo:d * 8192 + hi])

            # ---- stage A: conv-B -> v_nat [w, c, h] fp32r (+ ones col 128)
            # flat [128, 64*129 + 126] with tail pad so the last AV mm's
            # 256-wide moving slice stays in bounds.
            vflat = vbpool.tile([128, 64 * 129 + 128], fr)
            vbuf = vflat[:, 0:64 * 129].rearrange("p (c h) -> p c h", c=64)
            nc.gpsimd.memset(vflat[:, 64 * 129:].bitcast(f32), 0.0)
            nc.vector.tensor_copy(vbuf[:, :, 128], ones_f[:])
            for g in range(16):
                if g % 2 == 1:
                    pbt = pSC.tile([128, 512], f32, tag="psc", name="pbt")
                else:
                    pbt = pCB.tile([128, 512], f32, tag="pbt", name="pbt")
                pb = pbt[:].rearrange("p (a b) -> p a b", a=4)
                for t in range(4):
                    hp = 4 * g + t
                    nc.tensor.matmul(
                        pb[:, t, :],
                        xb_sb[:, hp * 128:(hp + 1) * 128],
                        wvbd_sb[:],
                        start=True, stop=True,
                    )
                # src [w, t(4), par(2), c(64)] -> dst vbuf[w, c, 8h]
                src = pb[:].rearrange("p t (r c) -> p c (t r)", r=2)
                if g % 2 == 0:
                    nc.vector.tensor_copy(vbuf[:, :, 8 * g:8 * g + 8], src)
                else:
                    nc.scalar.copy(vbuf[:, :, 8 * g:8 * g + 8], src)

            # ---- stage B: conv-A -> qbuf, kbuf fp16 [h, c, w]
            qbuf = qkpool.tile([128, 64, 128], fh, tag="q")
            kbuf = qkpool.tile([128, 64, 128], fh, tag="k")
            for g in range(32):
                if d == 0 and g % 2 == 1:
                    pat = pAV.tile([128, 512], f32, tag="pav", name="pat")
                else:
                    pat = pA.tile([128, 512], f32, tag="pat", name="pat")
                # [h, wp(2), par(2), qk*64]
                pa = pat[:].rearrange("p (a b c) -> p a b c", a=2, b=2)
                for i in range(2):
                    wp = 2 * g + i
                    nc.tensor.matmul(
                        pa[:, i, :, :].rearrange("p a b -> p (a b)"),
                        xa_sb[:, wp * 128:(wp + 1) * 128],
                        wbda_sb[:],
                        start=True, stop=True,
                    )
                # cols per parity: [q(64) | k(64)]
                # w index = 4g + 2i + par
                qsrc = pa[:, :, :, 0:64].rearrange("p i r c -> p c (i r)")
                ksrc = pa[:, :, :, 64:128].rearrange("p i r c -> p c (i r)")
                nc.vector.tensor_add(
                    qbuf[:, :, 4 * g:4 * g + 4], qsrc,
                    bq4_sb[:].rearrange("p a b c -> p c (a b)"),
                )
                nc.scalar.copy(kbuf[:, :, 4 * g:4 * g + 4], ksrc)

            if prev[0] is not None:
                stage_d(*prev[0])
                prev[0] = None

            # ---- stage C: attention, 4-channel score/exp quads
            obuf = obpool.tile([128, 128, 64], fh)   # [w, h, c]
            fv = vflat[:]
            for c4 in range(0, 64, 4):
                psc = pSC.tile([128, 512], f32)
                for j in range(4):
                    nc.tensor.matmul(
                        psc[:, j * 128:(j + 1) * 128],
                        kbuf[:, c4 + j, :], qbuf[:, c4 + j, :],
                        start=True, stop=True)
                if DEBUG and d == 0 and c4 == 0:
                    scf = ypool.tile([128, 512], f32, tag="dbgs")
                    nc.vector.tensor_copy(scf[:], psc[:])
                    nc.gpsimd.dma_start(dbg_sc[:], scf[:])
                et = etpool.tile([128, 4, 128], fr)
                nc.scalar.activation(
                    et[:], psc[:].rearrange("p (a b) -> p a b", a=4),
                    AF.Exp, bias=eshift_sb[:],
                )
                for jj in (0, 2):
                    c2 = c4 + jj
                    # AV: mm0 -> [0:258] (o_c2 + denom at 128), mm1 ->
                    # [256:512] with rhs starting at channel c2+1's block
                    # (o_c2+1 at 256:384, denom at 384): uniform stride.
                    pav = pAV.tile([128, 512], f32)
                    nc.tensor.matmul(
                        pav[:, 0:258], et[:, jj, :],
                        fv[:, c2 * 129:c2 * 129 + 258],
                        start=True, stop=True)
                    nc.tensor.matmul(
                        pav[:, 256:512], et[:, jj + 1, :],
                        fv[:, (c2 + 1) * 129:(c2 + 1) * 129 + 256],
                        start=True, stop=True)
                    pav2 = pav[:].rearrange("p (a b) -> p a b", a=2)
                    rr = rcpool.tile([128, 2], f32)
                    nc.vector.reciprocal(rr[:], pav2[:, :, 128])
                    # o evac: scale must happen before fp16 (underflow).
                    # Even pairs: direct DVE scale-evac. Odd pairs: stage to
                    # fp32 SBUF on ScalarE + DVE 2x-mode scale.
                    if jj == 0:
                        nc.vector.tensor_scalar_mul(
                            obuf[:, :, c2], pav2[:, 0, 0:128], rr[:, 0:1]
                        )
                        nc.vector.tensor_scalar_mul(
                            obuf[:, :, c2 + 1], pav2[:, 1, 0:128], rr[:, 1:2]
                        )
                    else:
                        ostg = ostgpool.tile([128, 2, 128], f32)
                        nc.scalar.copy(ostg[:], pav2[:, :, 0:128])
                        nc.vector.tensor_scalar_mul(
                            obuf[:, :, c2], ostg[:, 0, :], rr[:, 0:1]
                        )
                        nc.vector.tensor_scalar_mul(
                            obuf[:, :, c2 + 1], ostg[:, 1, :], rr[:, 1:2]
                        )

            if DEBUG and d == 0:
                nc.gpsimd.dma_start(
                    dbg_qk[:, 0:8192], qbuf[:].rearrange("p a b -> p (a b)"))
                nc.gpsimd.dma_start(
                    dbg_qk[:, 8192:16384], kbuf[:].rearrange("p a b -> p (a b)"))
                nc.gpsimd.dma_start(
                    dbg_vb[:], vbuf[:].rearrange("p a b -> p (a b)").bitcast(f32))
                nc.gpsimd.dma_start(
                    dbg_ob[:], obuf[:].rearrange("p a b -> p (a b)"))

            prev[0] = (d, obuf)

        if prev[0] is not None:
            stage_d(*prev[0], last=True)
            prev[0] = None

    nc.compile()
    return nc


def _prep_inputs(x, wq, bq, wk, bk, wv, bv, wo, bo):
    """Build per-core input maps."""
    x = np.asarray(x, _f32)[0]           # [64, 48, 128, 128]
    wq2 = np.asarray(wq, _f32) * QSCALE
    bq2 = np.asarray(bq, _f32) * QSCALE
    wk = np.asarray(wk, _f32)
    wv = np.asarray(wv, _f32); bv = np.asarray(bv, _f32)
    wo = np.asarray(wo, _f32); bo = np.asarray(bo, _f32)

    # conv-A weights: [c-pair, 2 x (q|k)]
    A = np.zeros((64, 128), _f32)
    A[:, 0:64] = wq2.T
    A[:, 64:128] = wk.T
    wbda = np.zeros((128, 256), _f32)
    wbda[0:64, 0:128] = A
    wbda[64:128, 128:256] = A
    wbda = round_fp32r(wbda)

    # conv-B weights fp16: blockdiag(wv^T)
    wvbd = np.zeros((128, 128), _f32)
    wvbd[0:64, 0:64] = wv.T
    wvbd[64:128, 64:128] = wv.T
    wvbd = wvbd.astype(_f16)

    bq4 = np.ascontiguousarray(
        np.broadcast_to(bq2[None, None, :], (128, 4, 64)).reshape(128, 256),
        dtype=_f32,
    )

    identh = np.eye(128, dtype=_f32).astype(_f16)

    # blockdiag wo^T (h-parity major): wobd2[64p+c, 64p+c'] = wo[c',c]
    wobd2 = np.zeros((128, 128), _f32)
    woT = wo.T  # [c, c']
    wobd2[0:64, 0:64] = woT
    wobd2[64:128, 64:128] = woT
    wobd2 = wobd2.astype(_f16)

    bo2v = (bo + wo @ bv).astype(_f32)
    bo2i = np.concatenate([bo2v, bo2v]).reshape(128, 1)

    in_maps = []
    for i in range(NCORES):
        xc = x[:, i * DLOC:(i + 1) * DLOC]          # [64, 6, 128, 128]
        xa = np.empty((128, DLOC, 64, 128), _f32)
        xa[0:64] = xc[:, :, 0::2, :]                # w even: [c, d, w2, h]
        xa[64:128] = xc[:, :, 1::2, :]              # w odd
        xa = round_fp32r(xa.reshape(128, DLOC * 64 * 128))
        xbv = np.empty((128, DLOC, 64, 128), _f16)
        xbv[0:64] = xc[:, :, :, 0::2].transpose(0, 1, 3, 2)   # [c, d, h2, w]
        xbv[64:128] = xc[:, :, :, 1::2].transpose(0, 1, 3, 2)
        xb = np.ascontiguousarray(xbv.reshape(128, DLOC * 64 * 128))
        in_maps.append({
            "xa": xa, "xb": xb, "wbda": wbda, "wvbd": wvbd, "bq4": bq4,
            "identh": identh, "wobd2": wobd2, "bo2i": bo2i,
        })
    return in_maps


def _decode_outputs(results):
    """results: per-core dicts with 'y' [DLOC, 4, 128, 2048]."""
    outs = []
    for r in results:
        y = np.asarray(r["y"], _f32).reshape(DLOC, 4, 2, 64, 4, 4, 128)
        # dims: [d, q, p, c', u, t, w]; h = 32q + 8u + 2t + p
        Y = np.transpose(y, (3, 0, 6, 1, 4, 5, 2)).reshape(64, DLOC, 128, 128)
        outs.append(Y)
    return np.concatenate(outs, axis=1)[None]        # [1, 64, 48, 128, 128]


_CACHE = {}


def _get_runner():
    """Build bass program + cached jitted pjrt callable."""
    if "runner" in _CACHE:
        return _CACHE["runner"]
    import jax
    import jax.numpy as jnp
    from jax.sharding import Mesh, PartitionSpec
    from jax.experimental.shard_map import shard_map
    import concourse.mybir as mybir
    from concourse import bass2jax
    from concourse.bass2jax import _bass_exec_p, install_neuronx_cc_hook

    install_neuronx_cc_hook()
    nc = _build_bass()

    partition_name = (
        nc.partition_id_tensor.name if nc.partition_id_tensor else None
    )
    in_names, out_names, out_avals = [], [], []
    for alloc in nc.m.functions[0].allocations:
        if not isinstance(alloc, mybir.MemoryLocationSet):
            continue
        name = alloc.memorylocations[0].name
        if alloc.kind == "ExternalInput":
            if name != partition_name:
                in_names.append(name)
        elif alloc.kind == "ExternalOutput":
            out_names.append(name)
            out_avals.append(
                jax.core.ShapedArray(
                    tuple(alloc.tensor_shape), mybir.dt.np(alloc.dtype)
                )
            )
    n_params = len(in_names)
    zero_shapes = [(a.shape, a.dtype) for a in out_avals]
    all_in_names = list(in_names) + list(out_names)
    if partition_name is not None:
        all_in_names.append(partition_name)

    def _body(*args):
        operands = list(args)
        if partition_name is not None:
            operands.append(bass2jax.partition_id_tensor())
        outs = _bass_exec_p.bind(
            *operands,
            out_avals=tuple(out_avals),
            in_names=tuple(all_in_names),
            out_names=tuple(out_names),
            lowering_input_output_aliases=(),
            sim_require_finite=True,
            sim_require_nnan=True,
            nc=nc,
        )
        return tuple(outs)

    devices = jax.devices()[:NCORES]
    mesh = Mesh(np.asarray(devices), ("core",))
    n_outs = len(out_names)
    in_specs = (PartitionSpec("core"),) * (n_params + n_outs)
    out_specs = (PartitionSpec("core"),) * n_outs
    donate = tuple(range(n_params, n_params + n_outs))
    sharded = jax.jit(
        shard_map(_body, mesh=mesh, in_specs=in_specs, out_specs=out_specs,
                  check_rep=False),
        donate_argnums=donate,
        keep_unused=True,
    )

    def run(in_maps):
        concat_in = [
            np.concatenate([np.asarray(in_maps[c][nm]) for c in range(NCORES)],
                           axis=0)
            for nm in in_names
        ]
        concat_zeros = [
            np.zeros((NCORES * s[0],) + tuple(s[1:]), dt)
            for (s, dt) in zero_shapes
        ]
        out = sharded(*concat_in, *concat_zeros)
        res = []
        for c in range(NCORES):
            res.append({
                nm: np.asarray(out[i]).reshape(NCORES, *zero_shapes[i][0])[c]
                for i, nm in enumerate(out_names)
            })
        return res, (sharded, in_names, zero_shapes, out_names)

    _CACHE["runner"] = run
    return run


def kernel(**inputs):
    run = _get_runner()
    in_maps = _prep_inputs(**inputs)
    results, _ = run(in_maps)
    return _decode_outputs(results)


if __name__ == "__main__":
    import reference
    t0 = time.time()
    ins = {k: np.asarray(v) for k, v in reference.setup_inputs().items()}
    exp = np.asarray(reference.reference(**ins))
    t1 = time.time()
    print(f"reference: {t1 - t0:.1f}s", flush=True)
    act = kernel(**ins)
    t2 = time.time()
    print(f"kernel: {t2 - t1:.1f}s", flush=True)
    err = np.abs(act - exp)
    scale = np.abs(exp).mean()
    print(f"abs err max={err.max():.3e} mean={err.mean():.3e} "
          f"rel(max/scale)={err.max() / scale:.3e} "
          f"rel_mean={(err / (np.abs(exp) + 1e-6)).mean():.3e}", flush=True)
